# revision 46
# baseline (speedup 1.0000x reference)
"""Dilated segment attention on 8 Trainium2 NeuronCores (Bass/Tile).

Problem: x:[4,8192,1024] fp32. Per 64-token segment, rows ::2 are kept
(32 tokens), projected with Wq/Wk/Wv (+bias), and full-dim attention is
computed within each segment. Output: [4,4096,1024] fp32.

Sharding: data-parallel. Core c handles batch c//2, sequence half c%2 ->
2048 dilated tokens = 64 segments. No collectives. The host passes the
weights transposed ([d_in, d_out] layout, a pure layout prep like the
bias reshape) and per-core contiguous x slices.

Per-core pipeline (all matmuls bf16, fp32 PSUM accumulation):
  - SWDGE cast-DMAs (fp32->bf16): W.T row-tiles straight into SBUF;
    the dilated x rows into DRAM scratch, then big xbar DMA-transposes
    (DRAM->SBUF, one per (512-token chunk, 128-d stripe)) build
    x.T [d_in, tok] in SBUF. A short junk-matmul warm-up keeps the PE
    HAM at 2.4 GHz while the first transfers land.
  - Weight-stationary passes -> q.T, k.T [d_out, tok] (bias fused into
    the ACT psum->sbuf epilogue), chunk-outer so compute starts on chunk
    0 while later chunks stream. x-stationary pass -> v [tok, d_out].
  - simT per 4-segment group as one packed 128x128 matmul over 8 k-tiles
    (diagonal 32x32 blocks are the real per-segment logits; logits are
    bounded ~[-1.6,1.6] so no max-subtraction is needed). ACT computes
    p = exp(scale*simT) from PSUM into a zeroed tile, diagonal blocks
    only, so full-width K=128 matmuls against p contract the off-diag
    zeros away.
  - attn@v and the softmax denominator l (ones-column matmul) per token
    tile; final out = psum_av * (1/l) + bv in one DVE
    scalar_tensor_tensor (v is projected without bias: softmax rows sum
    to 1, so + bv after).
"""

import numpy as np

P = 128
D = 1024
KT = 8  # d_in tiles of 128
OT = 8  # d_out tiles of 128
NTT = 16  # token tiles of 128 (2048 tokens per core)
FD = 512  # matmul moving free dim / psum bank
TCH = 4  # token chunks of 512

_CACHE = {}


def _build_nc():
    import os
    from contextlib import ExitStack

    import concourse.bass as bass
    import concourse.mybir as mybir
    import concourse.tile as tile
    from concourse import bacc

    KPHASE = int(os.environ.get("KPHASE", "5"))
    NWARM = int(os.environ.get("KWARM", "0"))

    dt = mybir.dt
    AF = mybir.ActivationFunctionType
    ALU = mybir.AluOpType

    nc = bacc.Bacc("TRN2", target_bir_lowering=False, debug=False,
                   enable_asserts=False)

    x_d = nc.dram_tensor("x", [4096, D], dt.float32, kind="ExternalInput")
    wqt_d = nc.dram_tensor("wqt", [D, D], dt.float32, kind="ExternalInput")
    wkt_d = nc.dram_tensor("wkt", [D, D], dt.float32, kind="ExternalInput")
    wvt_d = nc.dram_tensor("wvt", [D, D], dt.float32, kind="ExternalInput")
    bq_d = nc.dram_tensor("bqr", [P, OT], dt.float32, kind="ExternalInput")
    bk_d = nc.dram_tensor("bkr", [P, OT], dt.float32, kind="ExternalInput")
    bv_d = nc.dram_tensor("bvb", [1, D], dt.bfloat16, kind="ExternalInput")
    out_d = nc.dram_tensor("out", [2048, D], dt.float32, kind="ExternalOutput")

    wt_dram = [wqt_d, wkt_d, wvt_d]
    scale = float(D) ** -0.5

    with tile.TileContext(nc) as tc, ExitStack() as ctx:
        consts = ctx.enter_context(tc.tile_pool(name="consts", bufs=1))
        resid = ctx.enter_context(tc.tile_pool(name="resid", bufs=1))
        wtp = ctx.enter_context(tc.tile_pool(name="wtp", bufs=2))
        wstage = ctx.enter_context(tc.tile_pool(name="wstage", bufs=4))
        outp = ctx.enter_context(tc.tile_pool(name="outp", bufs=3))
        rsbp = ctx.enter_context(tc.tile_pool(name="rsbp", bufs=2))
        dpool = ctx.enter_context(tc.tile_pool(name="dram", bufs=1,
                                               space="DRAM"))

        ones_col = consts.tile([P, 1], dt.bfloat16, name="ones_col")
        ones_row = consts.tile([1, P], dt.bfloat16, name="ones_row")
        bq_sb = consts.tile([P, OT], dt.float32, name="bq_sb")
        bk_sb = consts.tile([P, OT], dt.float32, name="bk_sb")
        bvb_sb = consts.tile([1, D], dt.bfloat16, name="bvb_sb")
        bv_rep = consts.tile([P, D], dt.float32, name="bv_rep")

        nc.vector.memset(ones_col[:], 1.0)
        nc.vector.memset(ones_row[:], 1.0)
        nc.sync.dma_start(bq_sb[:], bq_d[:])
        nc.sync.dma_start(bk_sb[:], bk_d[:])
        nc.sync.dma_start(bvb_sb[:], bv_d[:])

        xT = [resid.tile([P, 2048], dt.bfloat16, name=f"xT{k}") for k in range(KT)]
        qT = [resid.tile([P, 2048], dt.bfloat16, name=f"qT{o}") for o in range(OT)]
        kT = [resid.tile([P, 2048], dt.bfloat16, name=f"kT{o}") for o in range(OT)]
        vv = [resid.tile([P, D], dt.bfloat16, name=f"v{t}") for t in range(NTT)]
        pT = [resid.tile([P, P], dt.bfloat16, name=f"pT{g}") for g in range(NTT)]

        # pT holds block-diagonal exp(sim) — zero once, exp writes only the
        # diagonal 32x32 blocks, so full-width (K=128) attn@v and l matmuls
        # contract the zeros away.
        for g in range(NTT):
            nc.vector.memset(pT[g][:], 0.0)

        # ---- x: dilated rows cast to bf16 DRAM scratch (SWDGE), then big
        # xbar transposes DRAM->SBUF build x.T. All transposes stay on the
        # sync HWDGE queue (concurrent xbar transposes from two queues
        # corrupt data — measured).
        x_bf = dpool.tile([2048, D], dt.bfloat16, name="x_bf")

        def cast_x_chunk(c):
            # 512 dilated tokens: 4 token tiles x (4 segs x 32 rows of ::2)
            src = bass.AP(x_d, 1024 * D * c,
                          [[256 * D, 4], [64 * D, 4], [2 * D, 32], [1, D]])
            return nc.gpsimd.dma_start(x_bf[512 * c:512 * c + 512, :], src)

        def load_wT(j, swdge=False):
            """W.T [d_in, d_out] bf16 row-tiles from the host-transposed
            weights. swdge=True: direct cast-DMA (half the bytes — used for
            Wq whose latency gates the start; the descriptor ring has room
            early). Otherwise: plain fp32 loads on the sync HWDGE queue
            (keeps the SWDGE ring free for the x casts) + ACT cast to bf16.
            Pool tags shared across passes reuse the same slots."""
            wts = []
            for i in range(KT):
                src = bass.AP(wt_dram[j], i * P * D, [[D, P], [1, D]])
                wt = wtp.tile([P, D], dt.bfloat16, name=f"wT{i}")
                if swdge:
                    nc.gpsimd.dma_start(wt[:], src)
                else:
                    stg = wstage.tile([P, D], dt.float32, name="wstg")
                    nc.sync.dma_start(stg[:], src)
                    nc.vector.tensor_copy(wt[:], stg[:])
                wts.append(wt)
            return wts

        def transpose_x_chunk(c):
            for k in range(KT):
                nc.sync.dma_start(xT[k][:, FD * c:FD * c + FD],
                                  x_bf[FD * c:FD * c + FD, P * k:P * k + P],
                                  transpose=True)

        # SWDGE order = demand order: x chunk 0, Wq.T tiles, then the rest.
        # xc2/xc3 are held behind xc0/xc1 completion so the critical first
        # chunk + Wq loads get the SDMA bandwidth (concurrent DMAs share it
        # round-robin).
        from concourse.bass import _add_dep_helper

        xc0 = cast_x_chunk(0)
        wq_ts = load_wT(0, swdge=True)
        xc1 = cast_x_chunk(1)
        xc2 = cast_x_chunk(2)
        _add_dep_helper(xc2.ins, xc0.ins, reason="throttle xc2 behind xc0")
        xc3 = cast_x_chunk(3)
        _add_dep_helper(xc3.ins, xc1.ins, reason="throttle xc3 behind xc1")
        transpose_x_chunk(0)

        if KPHASE >= 2:
            if NWARM:
                # HAM warm-up: full-K junk matmuls so real matmuls start at
                # 2.4 GHz. (K=1 fillers don't register as PE activity.)
                junk_w = consts.tile([P, P], dt.bfloat16, name="junk_w")
                junk_m = consts.tile([P, FD], dt.bfloat16, name="junk_m")
                nc.vector.memset(junk_w[:], 0.0)
                nc.vector.memset(junk_m[:], 0.0)
                with tc.tile_pool(name="warm", bufs=1, space="PSUM") as wp:
                    wps = wp.tile([P, FD], dt.float32, name="wps")
                    for _ in range(NWARM):
                        nc.tensor.matmul(wps[:], junk_w[:], junk_m[:],
                                         start=True, stop=True)

            with tc.tile_pool(name="ppool", bufs=6, space="PSUM") as ppool, \
                 tc.tile_pool(name="spool", bufs=2, space="PSUM") as spool:

                # ---- bv broadcast to all partitions via K=1 ones matmul
                for dh in range(2):
                    ps = ppool.tile([P, FD], dt.float32, name="pps")
                    nc.tensor.matmul(ps[:], ones_row[:],
                                     bvb_sb[:, FD * dh:FD * dh + FD],
                                     start=True, stop=True)
                    nc.scalar.copy(bv_rep[:, FD * dh:FD * dh + FD], ps[:])

                # ---- q/k passes: weights stationary, x.T moving -> q.T/k.T
                # chunk-outer so the PE starts on chunk 0 while later x
                # chunks are still being cast/transposed.
                def proj_pass(j, b_sb, dstT, wts=None):
                    if wts is None:
                        wts = load_wT(j)
                    if j == 0:
                        for c in range(1, TCH):
                            transpose_x_chunk(c)
                    for c in range(TCH):
                        for o in range(OT):
                            pss = ppool.tile([P, FD], dt.float32, name="pps")
                            for i in range(KT):
                                nc.tensor.matmul(pss[:],
                                                 wts[i][:, P * o:P * o + P],
                                                 xT[i][:, FD * c:FD * c + FD],
                                                 start=(i == 0),
                                                 stop=(i == KT - 1))
                            nc.scalar.activation(dstT[o][:, FD * c:FD * c + FD],
                                                 pss[:], AF.Identity,
                                                 bias=b_sb[:, o:o + 1],
                                                 scale=1.0)

                proj_pass(0, bq_sb, qT, wts=wq_ts)
                if KPHASE >= 3:
                    proj_pass(1, bk_sb, kT)

                    # ---- simT per 4-seg group; p = exp(scale*simT) via ACT
                    # (only the diagonal blocks — pT stays 0 elsewhere)
                    for g in range(NTT):
                        sps = spool.tile([P, P], dt.float32, name="sps")
                        for kk in range(KT):
                            nc.tensor.matmul(sps[:], kT[kk][:, P * g:P * g + P],
                                             qT[kk][:, P * g:P * g + P],
                                             start=(kk == 0),
                                             stop=(kk == KT - 1))
                        for a in range(4):
                            nc.scalar.activation(
                                pT[g][32 * a:32 * a + 32, 32 * a:32 * a + 32],
                                sps[32 * a:32 * a + 32, 32 * a:32 * a + 32],
                                AF.Exp, bias=0.0, scale=scale)

        if KPHASE >= 4:
            # ---- v pass interleaved with attn@v: AV group t only needs
            # vv[t] (just produced) and pT[t] (from the sim phase), so each
            # AV group hides behind the next v tile's matmuls instead of
            # running serially at the end. Block-diag pT makes the
            # full-width K=128 AV and l matmuls exact.
            with tc.tile_pool(name="vpool", bufs=4, space="PSUM") as vpool, \
                 tc.tile_pool(name="avp", bufs=3, space="PSUM") as avp, \
                 tc.tile_pool(name="lp", bufs=1, space="PSUM") as lp:
                wvs = load_wT(2)
                for t in range(NTT):
                    pss = [vpool.tile([P, FD], dt.float32, name="pps")
                           for _ in range(2)]
                    for i in range(KT):
                        for dh in range(2):
                            nc.tensor.matmul(pss[dh][:],
                                             xT[i][:, P * t:P * t + P],
                                             wvs[i][:, FD * dh:FD * dh + FD],
                                             start=(i == 0),
                                             stop=(i == KT - 1))
                    for dh in range(2):
                        nc.vector.tensor_copy(
                            vv[t][:, FD * dh:FD * dh + FD], pss[dh][:])
                    if KPHASE >= 5:
                        lps = lp.tile([P, 1], dt.float32, name="lps")
                        nc.tensor.matmul(lps[:], pT[t][:], ones_col[:],
                                         start=True, stop=True)
                        rsb = rsbp.tile([P, 1], dt.float32, name="rsb")
                        nc.vector.reciprocal(rsb[:], lps[:])
                        osb = outp.tile([P, D], dt.float32, name="osb")
                        for dh in range(2):
                            avs = avp.tile([P, FD], dt.float32, name="avs")
                            nc.tensor.matmul(avs[:], pT[t][:],
                                             vv[t][:, FD * dh:FD * dh + FD],
                                             start=True, stop=True)
                            nc.vector.scalar_tensor_tensor(
                                osb[:, FD * dh:FD * dh + FD], avs[:], rsb[:],
                                bv_rep[:, FD * dh:FD * dh + FD],
                                ALU.mult, ALU.add)
                        nc.sync.dma_start(
                            bass.AP(out_d, t * P * D, [[D, P], [1, D]]),
                            osb[:])
        if KPHASE < 5:
            dmp = outp.tile([P, D], dt.float32, name="osb")
            nc.vector.memset(dmp[:], 0.0)
            nc.sync.dma_start(bass.AP(out_d, 0, [[D, P], [1, D]]), dmp[:])

    nc.compile()
    return nc


def get_nc():
    if "nc" not in _CACHE:
        _CACHE["nc"] = _build_nc()
    return _CACHE["nc"]


def make_in_maps(x, Wq, bq, Wk, bk, Wv, bv):
    import ml_dtypes

    x = np.asarray(x, np.float32)
    wqt = np.ascontiguousarray(np.asarray(Wq, np.float32).T)
    wkt = np.ascontiguousarray(np.asarray(Wk, np.float32).T)
    wvt = np.ascontiguousarray(np.asarray(Wv, np.float32).T)
    bqr = np.ascontiguousarray(np.asarray(bq, np.float32).reshape(OT, P).T)
    bkr = np.ascontiguousarray(np.asarray(bk, np.float32).reshape(OT, P).T)
    bvb = np.asarray(bv, np.float32).reshape(1, D).astype(ml_dtypes.bfloat16)
    in_maps = []
    for c in range(8):
        b, h = divmod(c, 2)
        xs = np.ascontiguousarray(x[b, 4096 * h:4096 * h + 4096, :])
        in_maps.append({"x": xs, "wqt": wqt, "wkt": wkt, "wvt": wvt,
                        "bqr": bqr, "bkr": bkr, "bvb": bvb})
    return in_maps


def kernel(x, Wq, bq, Wk, bk, Wv, bv):
    from concourse.bass_utils import run_bass_kernel_spmd

    nc = get_nc()
    in_maps = make_in_maps(x, Wq, bq, Wk, bk, Wv, bv)
    res = run_bass_kernel_spmd(nc, in_maps, core_ids=list(range(8)))
    _CACHE["last_res"] = res
    out = np.empty((4, 4096, D), np.float32)
    for c in range(8):
        b, h = divmod(c, 2)
        out[b, 2048 * h:2048 * h + 2048] = res.results[c]["out"]
    return out


# revision 47
# speedup vs baseline: 1.0675x; 1.0675x over previous
"""Dilated segment attention on 8 Trainium2 NeuronCores (Bass/Tile).

Problem: x:[4,8192,1024] fp32. Per 64-token segment, rows ::2 are kept
(32 tokens), projected with Wq/Wk/Wv (+bias), and full-dim attention is
computed within each segment. Output: [4,4096,1024] fp32.

Sharding: data-parallel. Core c handles batch c//2, sequence half c%2 ->
2048 dilated tokens = 64 segments. No collectives. The host passes the
weights transposed ([d_in, d_out] layout, a pure layout prep like the
bias reshape) and per-core contiguous x slices.

Per-core pipeline (all matmuls bf16, fp32 PSUM accumulation):
  - SWDGE cast-DMAs (fp32->bf16): W.T row-tiles straight into SBUF;
    the dilated x rows into DRAM scratch, then big xbar DMA-transposes
    (DRAM->SBUF, one per (512-token chunk, 128-d stripe)) build
    x.T [d_in, tok] in SBUF. A short junk-matmul warm-up keeps the PE
    HAM at 2.4 GHz while the first transfers land.
  - Weight-stationary passes -> q.T, k.T [d_out, tok] (bias fused into
    the ACT psum->sbuf epilogue), chunk-outer so compute starts on chunk
    0 while later chunks stream. x-stationary pass -> v [tok, d_out].
  - simT per 4-segment group as one packed 128x128 matmul over 8 k-tiles
    (diagonal 32x32 blocks are the real per-segment logits; logits are
    bounded ~[-1.6,1.6] so no max-subtraction is needed). ACT computes
    p = exp(scale*simT) from PSUM into a zeroed tile, diagonal blocks
    only, so full-width K=128 matmuls against p contract the off-diag
    zeros away.
  - attn@v and the softmax denominator l (ones-column matmul) per token
    tile; final out = psum_av * (1/l) + bv in one DVE
    scalar_tensor_tensor (v is projected without bias: softmax rows sum
    to 1, so + bv after).
"""

import numpy as np

P = 128
D = 1024
KT = 8  # d_in tiles of 128
OT = 8  # d_out tiles of 128
NTT = 16  # token tiles of 128 (2048 tokens per core)
FD = 512  # matmul moving free dim / psum bank
TCH = 4  # token chunks of 512

_CACHE = {}


def _build_nc():
    import os
    from contextlib import ExitStack

    import concourse.bass as bass
    import concourse.mybir as mybir
    import concourse.tile as tile
    from concourse import bacc

    KPHASE = int(os.environ.get("KPHASE", "5"))
    NWARM = int(os.environ.get("KWARM", "0"))

    dt = mybir.dt
    AF = mybir.ActivationFunctionType
    ALU = mybir.AluOpType

    nc = bacc.Bacc("TRN2", target_bir_lowering=False, debug=False,
                   enable_asserts=False)

    x_d = nc.dram_tensor("x", [4096, D], dt.float32, kind="ExternalInput")
    wqt_d = nc.dram_tensor("wqt", [D, D], dt.float32, kind="ExternalInput")
    wkt_d = nc.dram_tensor("wkt", [D, D], dt.float32, kind="ExternalInput")
    wvt_d = nc.dram_tensor("wvt", [D, D], dt.float32, kind="ExternalInput")
    bq_d = nc.dram_tensor("bqr", [P, OT], dt.float32, kind="ExternalInput")
    bk_d = nc.dram_tensor("bkr", [P, OT], dt.float32, kind="ExternalInput")
    bv_d = nc.dram_tensor("bvb", [1, D], dt.bfloat16, kind="ExternalInput")
    out_d = nc.dram_tensor("out", [2048, D], dt.float32, kind="ExternalOutput")

    wt_dram = [wqt_d, wkt_d, wvt_d]
    scale = float(D) ** -0.5

    with tile.TileContext(nc) as tc, ExitStack() as ctx:
        consts = ctx.enter_context(tc.tile_pool(name="consts", bufs=1))
        resid = ctx.enter_context(tc.tile_pool(name="resid", bufs=1))
        wtp = ctx.enter_context(tc.tile_pool(name="wtp", bufs=2))
        wstage = ctx.enter_context(tc.tile_pool(name="wstage", bufs=4))
        outp = ctx.enter_context(tc.tile_pool(name="outp", bufs=3))
        rsbp = ctx.enter_context(tc.tile_pool(name="rsbp", bufs=2))
        dpool = ctx.enter_context(tc.tile_pool(name="dram", bufs=1,
                                               space="DRAM"))

        ones_col = consts.tile([P, 1], dt.bfloat16, name="ones_col")
        ones_row = consts.tile([1, P], dt.bfloat16, name="ones_row")
        bq_sb = consts.tile([P, OT], dt.float32, name="bq_sb")
        bk_sb = consts.tile([P, OT], dt.float32, name="bk_sb")
        bvb_sb = consts.tile([1, D], dt.bfloat16, name="bvb_sb")
        bv_rep = consts.tile([P, D], dt.float32, name="bv_rep")

        nc.vector.memset(ones_col[:], 1.0)
        nc.vector.memset(ones_row[:], 1.0)
        nc.sync.dma_start(bq_sb[:], bq_d[:])
        nc.sync.dma_start(bk_sb[:], bk_d[:])
        nc.sync.dma_start(bvb_sb[:], bv_d[:])

        xT = [resid.tile([P, 2048], dt.bfloat16, name=f"xT{k}") for k in range(KT)]
        qT = [resid.tile([P, 2048], dt.bfloat16, name=f"qT{o}") for o in range(OT)]
        kT = [resid.tile([P, 2048], dt.bfloat16, name=f"kT{o}") for o in range(OT)]
        vv = [resid.tile([P, D], dt.bfloat16, name=f"v{t}") for t in range(NTT)]
        pT = [resid.tile([P, P], dt.bfloat16, name=f"pT{g}") for g in range(NTT)]

        # pT holds block-diagonal exp(sim) — zero once, exp writes only the
        # diagonal 32x32 blocks, so full-width (K=128) attn@v and l matmuls
        # contract the zeros away.
        for g in range(NTT):
            nc.vector.memset(pT[g][:], 0.0)

        # ---- x: dilated rows cast to bf16 DRAM scratch (SWDGE), then big
        # xbar transposes DRAM->SBUF build x.T. All transposes stay on the
        # sync HWDGE queue (concurrent xbar transposes from two queues
        # corrupt data — measured).
        x_bf = dpool.tile([2048, D], dt.bfloat16, name="x_bf")

        def cast_x_chunk(c):
            # 512 dilated tokens: 4 token tiles x (4 segs x 32 rows of ::2)
            src = bass.AP(x_d, 1024 * D * c,
                          [[256 * D, 4], [64 * D, 4], [2 * D, 32], [1, D]])
            return nc.gpsimd.dma_start(x_bf[512 * c:512 * c + 512, :], src)

        def load_wT(j, swdge=False):
            """W.T [d_in, d_out] bf16 row-tiles from the host-transposed
            weights. swdge=True: direct cast-DMA (half the bytes — used for
            Wq whose latency gates the start; the descriptor ring has room
            early). Otherwise: plain fp32 loads on the sync HWDGE queue
            (keeps the SWDGE ring free for the x casts) + ACT cast to bf16.
            Pool tags shared across passes reuse the same slots."""
            wts = []
            for i in range(KT):
                src = bass.AP(wt_dram[j], i * P * D, [[D, P], [1, D]])
                wt = wtp.tile([P, D], dt.bfloat16, name=f"wT{i}")
                if swdge:
                    nc.gpsimd.dma_start(wt[:], src)
                else:
                    stg = wstage.tile([P, D], dt.float32, name="wstg")
                    nc.sync.dma_start(stg[:], src)
                    nc.vector.tensor_copy(wt[:], stg[:])
                wts.append(wt)
            return wts

        def transpose_x_chunk(c):
            for k in range(KT):
                nc.sync.dma_start(xT[k][:, FD * c:FD * c + FD],
                                  x_bf[FD * c:FD * c + FD, P * k:P * k + P],
                                  transpose=True)

        # SWDGE order = demand order: x chunk 0, Wq.T tiles, then the rest.
        # xc2/xc3 are held behind xc0/xc1 completion so the critical first
        # chunk + Wq loads get the SDMA bandwidth (concurrent DMAs share it
        # round-robin).
        from concourse.bass import _add_dep_helper

        xc0 = cast_x_chunk(0)
        wq_ts = load_wT(0)
        xc1 = cast_x_chunk(1)
        xc2 = cast_x_chunk(2)
        _add_dep_helper(xc2.ins, xc0.ins, reason="throttle xc2 behind xc0")
        xc3 = cast_x_chunk(3)
        _add_dep_helper(xc3.ins, xc1.ins, reason="throttle xc3 behind xc1")
        transpose_x_chunk(0)

        if KPHASE >= 2:
            if NWARM:
                # HAM warm-up: full-K junk matmuls so real matmuls start at
                # 2.4 GHz. (K=1 fillers don't register as PE activity.)
                junk_w = consts.tile([P, P], dt.bfloat16, name="junk_w")
                junk_m = consts.tile([P, FD], dt.bfloat16, name="junk_m")
                nc.vector.memset(junk_w[:], 0.0)
                nc.vector.memset(junk_m[:], 0.0)
                with tc.tile_pool(name="warm", bufs=1, space="PSUM") as wp:
                    wps = wp.tile([P, FD], dt.float32, name="wps")
                    for _ in range(NWARM):
                        nc.tensor.matmul(wps[:], junk_w[:], junk_m[:],
                                         start=True, stop=True)

            with tc.tile_pool(name="ppool", bufs=6, space="PSUM") as ppool, \
                 tc.tile_pool(name="spool", bufs=2, space="PSUM") as spool:

                # ---- bv broadcast to all partitions via K=1 ones matmul
                for dh in range(2):
                    ps = ppool.tile([P, FD], dt.float32, name="pps")
                    nc.tensor.matmul(ps[:], ones_row[:],
                                     bvb_sb[:, FD * dh:FD * dh + FD],
                                     start=True, stop=True)
                    nc.scalar.copy(bv_rep[:, FD * dh:FD * dh + FD], ps[:])

                # ---- q/k passes: weights stationary, x.T moving -> q.T/k.T
                # chunk-outer so the PE starts on chunk 0 while later x
                # chunks are still being cast/transposed.
                def proj_pass(j, b_sb, dstT, wts=None):
                    if wts is None:
                        wts = load_wT(j)
                    if j == 0:
                        for c in range(1, TCH):
                            transpose_x_chunk(c)
                    for c in range(TCH):
                        for o in range(OT):
                            pss = ppool.tile([P, FD], dt.float32, name="pps")
                            for i in range(KT):
                                nc.tensor.matmul(pss[:],
                                                 wts[i][:, P * o:P * o + P],
                                                 xT[i][:, FD * c:FD * c + FD],
                                                 start=(i == 0),
                                                 stop=(i == KT - 1))
                            nc.scalar.activation(dstT[o][:, FD * c:FD * c + FD],
                                                 pss[:], AF.Identity,
                                                 bias=b_sb[:, o:o + 1],
                                                 scale=1.0)

                proj_pass(0, bq_sb, qT, wts=wq_ts)
                if KPHASE >= 3:
                    proj_pass(1, bk_sb, kT)

                    # ---- simT per 4-seg group; p = exp(scale*simT) via ACT
                    # (only the diagonal blocks — pT stays 0 elsewhere)
                    for g in range(NTT):
                        sps = spool.tile([P, P], dt.float32, name="sps")
                        for kk in range(KT):
                            nc.tensor.matmul(sps[:], kT[kk][:, P * g:P * g + P],
                                             qT[kk][:, P * g:P * g + P],
                                             start=(kk == 0),
                                             stop=(kk == KT - 1))
                        for a in range(4):
                            nc.scalar.activation(
                                pT[g][32 * a:32 * a + 32, 32 * a:32 * a + 32],
                                sps[32 * a:32 * a + 32, 32 * a:32 * a + 32],
                                AF.Exp, bias=0.0, scale=scale)

        if KPHASE >= 4:
            # ---- v pass interleaved with attn@v: AV group t only needs
            # vv[t] (just produced) and pT[t] (from the sim phase), so each
            # AV group hides behind the next v tile's matmuls instead of
            # running serially at the end. Block-diag pT makes the
            # full-width K=128 AV and l matmuls exact.
            with tc.tile_pool(name="vpool", bufs=4, space="PSUM") as vpool, \
                 tc.tile_pool(name="avp", bufs=3, space="PSUM") as avp, \
                 tc.tile_pool(name="lp", bufs=1, space="PSUM") as lp:
                wvs = load_wT(2)
                for t in range(NTT):
                    pss = [vpool.tile([P, FD], dt.float32, name="pps")
                           for _ in range(2)]
                    for i in range(KT):
                        for dh in range(2):
                            nc.tensor.matmul(pss[dh][:],
                                             xT[i][:, P * t:P * t + P],
                                             wvs[i][:, FD * dh:FD * dh + FD],
                                             start=(i == 0),
                                             stop=(i == KT - 1))
                    for dh in range(2):
                        nc.vector.tensor_copy(
                            vv[t][:, FD * dh:FD * dh + FD], pss[dh][:])
                    if KPHASE >= 5:
                        lps = lp.tile([P, 1], dt.float32, name="lps")
                        nc.tensor.matmul(lps[:], pT[t][:], ones_col[:],
                                         start=True, stop=True)
                        rsb = rsbp.tile([P, 1], dt.float32, name="rsb")
                        nc.vector.reciprocal(rsb[:], lps[:])
                        osb = outp.tile([P, D], dt.float32, name="osb")
                        for dh in range(2):
                            avs = avp.tile([P, FD], dt.float32, name="avs")
                            nc.tensor.matmul(avs[:], pT[t][:],
                                             vv[t][:, FD * dh:FD * dh + FD],
                                             start=True, stop=True)
                            nc.vector.scalar_tensor_tensor(
                                osb[:, FD * dh:FD * dh + FD], avs[:], rsb[:],
                                bv_rep[:, FD * dh:FD * dh + FD],
                                ALU.mult, ALU.add)
                        nc.sync.dma_start(
                            bass.AP(out_d, t * P * D, [[D, P], [1, D]]),
                            osb[:])
        if KPHASE < 5:
            dmp = outp.tile([P, D], dt.float32, name="osb")
            nc.vector.memset(dmp[:], 0.0)
            nc.sync.dma_start(bass.AP(out_d, 0, [[D, P], [1, D]]), dmp[:])

    nc.compile()
    return nc


def get_nc():
    if "nc" not in _CACHE:
        _CACHE["nc"] = _build_nc()
    return _CACHE["nc"]


def make_in_maps(x, Wq, bq, Wk, bk, Wv, bv):
    import ml_dtypes

    x = np.asarray(x, np.float32)
    wqt = np.ascontiguousarray(np.asarray(Wq, np.float32).T)
    wkt = np.ascontiguousarray(np.asarray(Wk, np.float32).T)
    wvt = np.ascontiguousarray(np.asarray(Wv, np.float32).T)
    bqr = np.ascontiguousarray(np.asarray(bq, np.float32).reshape(OT, P).T)
    bkr = np.ascontiguousarray(np.asarray(bk, np.float32).reshape(OT, P).T)
    bvb = np.asarray(bv, np.float32).reshape(1, D).astype(ml_dtypes.bfloat16)
    in_maps = []
    for c in range(8):
        b, h = divmod(c, 2)
        xs = np.ascontiguousarray(x[b, 4096 * h:4096 * h + 4096, :])
        in_maps.append({"x": xs, "wqt": wqt, "wkt": wkt, "wvt": wvt,
                        "bqr": bqr, "bkr": bkr, "bvb": bvb})
    return in_maps


def kernel(x, Wq, bq, Wk, bk, Wv, bv):
    from concourse.bass_utils import run_bass_kernel_spmd

    nc = get_nc()
    in_maps = make_in_maps(x, Wq, bq, Wk, bk, Wv, bv)
    res = run_bass_kernel_spmd(nc, in_maps, core_ids=list(range(8)))
    _CACHE["last_res"] = res
    out = np.empty((4, 4096, D), np.float32)
    for c in range(8):
        b, h = divmod(c, 2)
        out[b, 2048 * h:2048 * h + 2048] = res.results[c]["out"]
    return out


# revision 48
# speedup vs baseline: 1.0846x; 1.0160x over previous
"""Dilated segment attention on 8 Trainium2 NeuronCores (Bass/Tile).

Problem: x:[4,8192,1024] fp32. Per 64-token segment, rows ::2 are kept
(32 tokens), projected with Wq/Wk/Wv (+bias), and full-dim attention is
computed within each segment. Output: [4,4096,1024] fp32.

Sharding: data-parallel. Core c handles batch c//2, sequence half c%2 ->
2048 dilated tokens = 64 segments. No collectives. The host passes the
weights transposed ([d_in, d_out] layout, a pure layout prep like the
bias reshape) and per-core contiguous x slices.

Per-core pipeline (all matmuls bf16, fp32 PSUM accumulation):
  - SWDGE cast-DMAs (fp32->bf16): W.T row-tiles straight into SBUF;
    the dilated x rows into DRAM scratch, then big xbar DMA-transposes
    (DRAM->SBUF, one per (512-token chunk, 128-d stripe)) build
    x.T [d_in, tok] in SBUF. A short junk-matmul warm-up keeps the PE
    HAM at 2.4 GHz while the first transfers land.
  - Weight-stationary passes -> q.T, k.T [d_out, tok] (bias fused into
    the ACT psum->sbuf epilogue), chunk-outer so compute starts on chunk
    0 while later chunks stream. x-stationary pass -> v [tok, d_out].
  - simT per 4-segment group as one packed 128x128 matmul over 8 k-tiles
    (diagonal 32x32 blocks are the real per-segment logits; logits are
    bounded ~[-1.6,1.6] so no max-subtraction is needed). ACT computes
    p = exp(scale*simT) from PSUM into a zeroed tile, diagonal blocks
    only, so full-width K=128 matmuls against p contract the off-diag
    zeros away.
  - attn@v and the softmax denominator l (ones-column matmul) per token
    tile; final out = psum_av * (1/l) + bv in one DVE
    scalar_tensor_tensor (v is projected without bias: softmax rows sum
    to 1, so + bv after).
"""

import numpy as np

P = 128
D = 1024
KT = 8  # d_in tiles of 128
OT = 8  # d_out tiles of 128
NTT = 16  # token tiles of 128 (2048 tokens per core)
FD = 512  # matmul moving free dim / psum bank
TCH = 4  # token chunks of 512

_CACHE = {}


def _build_nc():
    import os
    from contextlib import ExitStack

    import concourse.bass as bass
    import concourse.mybir as mybir
    import concourse.tile as tile
    from concourse import bacc

    KPHASE = int(os.environ.get("KPHASE", "5"))
    NWARM = int(os.environ.get("KWARM", "0"))

    dt = mybir.dt
    AF = mybir.ActivationFunctionType
    ALU = mybir.AluOpType

    nc = bacc.Bacc("TRN2", target_bir_lowering=False, debug=False,
                   enable_asserts=False)

    x_d = nc.dram_tensor("x", [4096, D], dt.float32, kind="ExternalInput")
    wqt_d = nc.dram_tensor("wqt", [D, D], dt.float32, kind="ExternalInput")
    wkt_d = nc.dram_tensor("wkt", [D, D], dt.float32, kind="ExternalInput")
    wvt_d = nc.dram_tensor("wvt", [D, D], dt.float32, kind="ExternalInput")
    bq_d = nc.dram_tensor("bqr", [P, OT], dt.float32, kind="ExternalInput")
    bk_d = nc.dram_tensor("bkr", [P, OT], dt.float32, kind="ExternalInput")
    bv_d = nc.dram_tensor("bvb", [1, D], dt.bfloat16, kind="ExternalInput")
    out_d = nc.dram_tensor("out", [2048, D], dt.float32, kind="ExternalOutput")

    wt_dram = [wqt_d, wkt_d, wvt_d]
    scale = float(D) ** -0.5

    with tile.TileContext(nc) as tc, ExitStack() as ctx:
        consts = ctx.enter_context(tc.tile_pool(name="consts", bufs=1))
        resid = ctx.enter_context(tc.tile_pool(name="resid", bufs=1))
        wtp = ctx.enter_context(tc.tile_pool(name="wtp", bufs=2))
        wstage = ctx.enter_context(tc.tile_pool(name="wstage", bufs=4))
        outp = ctx.enter_context(tc.tile_pool(name="outp", bufs=3))
        rsbp = ctx.enter_context(tc.tile_pool(name="rsbp", bufs=2))
        dpool = ctx.enter_context(tc.tile_pool(name="dram", bufs=1,
                                               space="DRAM"))

        ones_col = consts.tile([P, 1], dt.bfloat16, name="ones_col")
        ones_row = consts.tile([1, P], dt.bfloat16, name="ones_row")
        bq_sb = consts.tile([P, OT], dt.float32, name="bq_sb")
        bk_sb = consts.tile([P, OT], dt.float32, name="bk_sb")
        bvb_sb = consts.tile([1, D], dt.bfloat16, name="bvb_sb")
        bv_rep = consts.tile([P, D], dt.float32, name="bv_rep")

        nc.vector.memset(ones_col[:], 1.0)
        nc.vector.memset(ones_row[:], 1.0)
        nc.sync.dma_start(bq_sb[:], bq_d[:])
        nc.sync.dma_start(bk_sb[:], bk_d[:])
        nc.sync.dma_start(bvb_sb[:], bv_d[:])

        xT = [resid.tile([P, 2048], dt.bfloat16, name=f"xT{k}") for k in range(KT)]
        qT = [resid.tile([P, 2048], dt.bfloat16, name=f"qT{o}") for o in range(OT)]
        kT = [resid.tile([P, 2048], dt.bfloat16, name=f"kT{o}") for o in range(OT)]
        vv = [resid.tile([P, D], dt.bfloat16, name=f"v{t}") for t in range(NTT)]
        pT = [resid.tile([P, P], dt.bfloat16, name=f"pT{g}") for g in range(NTT)]

        # pT holds block-diagonal exp(sim) — zero once, exp writes only the
        # diagonal 32x32 blocks, so full-width (K=128) attn@v and l matmuls
        # contract the zeros away.
        for g in range(NTT):
            nc.vector.memset(pT[g][:], 0.0)

        # ---- x: dilated rows cast to bf16 DRAM scratch (SWDGE), then big
        # xbar transposes DRAM->SBUF build x.T. All transposes stay on the
        # sync HWDGE queue (concurrent xbar transposes from two queues
        # corrupt data — measured).
        x_bf = dpool.tile([2048, D], dt.bfloat16, name="x_bf")

        def cast_x_chunk(c):
            # 512 dilated tokens: 4 token tiles x (4 segs x 32 rows of ::2)
            src = bass.AP(x_d, 1024 * D * c,
                          [[256 * D, 4], [64 * D, 4], [2 * D, 32], [1, D]])
            return nc.gpsimd.dma_start(x_bf[512 * c:512 * c + 512, :], src)

        def load_wT(j, swdge=False):
            """W.T [d_in, d_out] bf16 row-tiles from the host-transposed
            weights. swdge=True: direct cast-DMA (half the bytes — used for
            Wq whose latency gates the start; the descriptor ring has room
            early). Otherwise: plain fp32 loads on the sync HWDGE queue
            (keeps the SWDGE ring free for the x casts) + ACT cast to bf16.
            Pool tags shared across passes reuse the same slots."""
            wts = []
            for i in range(KT):
                src = bass.AP(wt_dram[j], i * P * D, [[D, P], [1, D]])
                wt = wtp.tile([P, D], dt.bfloat16, name=f"wT{i}")
                if swdge:
                    nc.gpsimd.dma_start(wt[:], src)
                else:
                    stg = wstage.tile([P, D], dt.float32, name="wstg")
                    nc.sync.dma_start(stg[:], src)
                    nc.vector.tensor_copy(wt[:], stg[:])
                wts.append(wt)
            return wts

        def transpose_x_chunk(c):
            for k in range(KT):
                nc.sync.dma_start(xT[k][:, FD * c:FD * c + FD],
                                  x_bf[FD * c:FD * c + FD, P * k:P * k + P],
                                  transpose=True)

        # SWDGE order = demand order: x chunk 0, Wq.T tiles, then the rest.
        # xc2/xc3 are held behind xc0/xc1 completion so the critical first
        # chunk + Wq loads get the SDMA bandwidth (concurrent DMAs share it
        # round-robin).
        from concourse.bass import _add_dep_helper

        xc0 = cast_x_chunk(0)
        wq_ts = load_wT(0)
        xc1 = cast_x_chunk(1)
        _add_dep_helper(xc1.ins, xc0.ins, reason="throttle xc1 behind xc0")
        xc2 = cast_x_chunk(2)
        _add_dep_helper(xc2.ins, xc0.ins, reason="throttle xc2 behind xc0")
        xc3 = cast_x_chunk(3)
        _add_dep_helper(xc3.ins, xc1.ins, reason="throttle xc3 behind xc1")
        transpose_x_chunk(0)

        if KPHASE >= 2:
            if NWARM:
                # HAM warm-up: full-K junk matmuls so real matmuls start at
                # 2.4 GHz. (K=1 fillers don't register as PE activity.)
                junk_w = consts.tile([P, P], dt.bfloat16, name="junk_w")
                junk_m = consts.tile([P, FD], dt.bfloat16, name="junk_m")
                nc.vector.memset(junk_w[:], 0.0)
                nc.vector.memset(junk_m[:], 0.0)
                with tc.tile_pool(name="warm", bufs=1, space="PSUM") as wp:
                    wps = wp.tile([P, FD], dt.float32, name="wps")
                    for _ in range(NWARM):
                        nc.tensor.matmul(wps[:], junk_w[:], junk_m[:],
                                         start=True, stop=True)

            with tc.tile_pool(name="ppool", bufs=6, space="PSUM") as ppool, \
                 tc.tile_pool(name="spool", bufs=2, space="PSUM") as spool:

                # ---- bv broadcast to all partitions via K=1 ones matmul
                for dh in range(2):
                    ps = ppool.tile([P, FD], dt.float32, name="pps")
                    nc.tensor.matmul(ps[:], ones_row[:],
                                     bvb_sb[:, FD * dh:FD * dh + FD],
                                     start=True, stop=True)
                    nc.scalar.copy(bv_rep[:, FD * dh:FD * dh + FD], ps[:])

                # ---- q/k passes: weights stationary, x.T moving -> q.T/k.T
                # chunk-outer so the PE starts on chunk 0 while later x
                # chunks are still being cast/transposed.
                def proj_pass(j, b_sb, dstT, wts=None):
                    if wts is None:
                        wts = load_wT(j)
                    if j == 0:
                        for c in range(1, TCH):
                            transpose_x_chunk(c)
                    for c in range(TCH):
                        for o in range(OT):
                            pss = ppool.tile([P, FD], dt.float32, name="pps")
                            for i in range(KT):
                                nc.tensor.matmul(pss[:],
                                                 wts[i][:, P * o:P * o + P],
                                                 xT[i][:, FD * c:FD * c + FD],
                                                 start=(i == 0),
                                                 stop=(i == KT - 1))
                            nc.scalar.activation(dstT[o][:, FD * c:FD * c + FD],
                                                 pss[:], AF.Identity,
                                                 bias=b_sb[:, o:o + 1],
                                                 scale=1.0)

                proj_pass(0, bq_sb, qT, wts=wq_ts)
                if KPHASE >= 3:
                    proj_pass(1, bk_sb, kT)

                    # ---- simT per 4-seg group; p = exp(scale*simT) via ACT
                    # (only the diagonal blocks — pT stays 0 elsewhere)
                    for g in range(NTT):
                        sps = spool.tile([P, P], dt.float32, name="sps")
                        for kk in range(KT):
                            nc.tensor.matmul(sps[:], kT[kk][:, P * g:P * g + P],
                                             qT[kk][:, P * g:P * g + P],
                                             start=(kk == 0),
                                             stop=(kk == KT - 1))
                        for a in range(4):
                            nc.scalar.activation(
                                pT[g][32 * a:32 * a + 32, 32 * a:32 * a + 32],
                                sps[32 * a:32 * a + 32, 32 * a:32 * a + 32],
                                AF.Exp, bias=0.0, scale=scale)

        if KPHASE >= 4:
            # ---- v pass interleaved with attn@v: AV group t only needs
            # vv[t] (just produced) and pT[t] (from the sim phase), so each
            # AV group hides behind the next v tile's matmuls instead of
            # running serially at the end. Block-diag pT makes the
            # full-width K=128 AV and l matmuls exact.
            with tc.tile_pool(name="vpool", bufs=4, space="PSUM") as vpool, \
                 tc.tile_pool(name="avp", bufs=3, space="PSUM") as avp, \
                 tc.tile_pool(name="lp", bufs=1, space="PSUM") as lp:
                wvs = load_wT(2)
                for t in range(NTT):
                    pss = [vpool.tile([P, FD], dt.float32, name="pps")
                           for _ in range(2)]
                    for i in range(KT):
                        for dh in range(2):
                            nc.tensor.matmul(pss[dh][:],
                                             xT[i][:, P * t:P * t + P],
                                             wvs[i][:, FD * dh:FD * dh + FD],
                                             start=(i == 0),
                                             stop=(i == KT - 1))
                    for dh in range(2):
                        nc.vector.tensor_copy(
                            vv[t][:, FD * dh:FD * dh + FD], pss[dh][:])
                    if KPHASE >= 5:
                        lps = lp.tile([P, 1], dt.float32, name="lps")
                        nc.tensor.matmul(lps[:], pT[t][:], ones_col[:],
                                         start=True, stop=True)
                        rsb = rsbp.tile([P, 1], dt.float32, name="rsb")
                        nc.vector.reciprocal(rsb[:], lps[:])
                        osb = outp.tile([P, D], dt.float32, name="osb")
                        for dh in range(2):
                            avs = avp.tile([P, FD], dt.float32, name="avs")
                            nc.tensor.matmul(avs[:], pT[t][:],
                                             vv[t][:, FD * dh:FD * dh + FD],
                                             start=True, stop=True)
                            nc.vector.scalar_tensor_tensor(
                                osb[:, FD * dh:FD * dh + FD], avs[:], rsb[:],
                                bv_rep[:, FD * dh:FD * dh + FD],
                                ALU.mult, ALU.add)
                        nc.sync.dma_start(
                            bass.AP(out_d, t * P * D, [[D, P], [1, D]]),
                            osb[:])
        if KPHASE < 5:
            dmp = outp.tile([P, D], dt.float32, name="osb")
            nc.vector.memset(dmp[:], 0.0)
            nc.sync.dma_start(bass.AP(out_d, 0, [[D, P], [1, D]]), dmp[:])

    nc.compile()
    return nc


def get_nc():
    if "nc" not in _CACHE:
        _CACHE["nc"] = _build_nc()
    return _CACHE["nc"]


def make_in_maps(x, Wq, bq, Wk, bk, Wv, bv):
    import ml_dtypes

    x = np.asarray(x, np.float32)
    wqt = np.ascontiguousarray(np.asarray(Wq, np.float32).T)
    wkt = np.ascontiguousarray(np.asarray(Wk, np.float32).T)
    wvt = np.ascontiguousarray(np.asarray(Wv, np.float32).T)
    bqr = np.ascontiguousarray(np.asarray(bq, np.float32).reshape(OT, P).T)
    bkr = np.ascontiguousarray(np.asarray(bk, np.float32).reshape(OT, P).T)
    bvb = np.asarray(bv, np.float32).reshape(1, D).astype(ml_dtypes.bfloat16)
    in_maps = []
    for c in range(8):
        b, h = divmod(c, 2)
        xs = np.ascontiguousarray(x[b, 4096 * h:4096 * h + 4096, :])
        in_maps.append({"x": xs, "wqt": wqt, "wkt": wkt, "wvt": wvt,
                        "bqr": bqr, "bkr": bkr, "bvb": bvb})
    return in_maps


def kernel(x, Wq, bq, Wk, bk, Wv, bv):
    from concourse.bass_utils import run_bass_kernel_spmd

    nc = get_nc()
    in_maps = make_in_maps(x, Wq, bq, Wk, bk, Wv, bv)
    res = run_bass_kernel_spmd(nc, in_maps, core_ids=list(range(8)))
    _CACHE["last_res"] = res
    out = np.empty((4, 4096, D), np.float32)
    for c in range(8):
        b, h = divmod(c, 2)
        out[b, 2048 * h:2048 * h + 2048] = res.results[c]["out"]
    return out
